# revision 17
# baseline (speedup 1.0000x reference)
"""Batched 1D Darcy solver (tridiagonal K shared across the batch) on 8
Trainium2 NeuronCores.

Math.  The reference assembles a CONSTANT tridiagonal matrix K (it depends
only on n=512 and AMPLITUDE=0.1) and solves K u = f where the RHS
f = assemble(forcing) is affine in the input:
    f[:, 1:-1] = forcing[:, 1:-1] * h/2,  f[:, 0] = 0,  f[:, -1] = sin(pi_f32)
Because K is constant, the whole solve collapses to one affine map,
precomputed on host in float64:

    u = forcing @ G' + ones(B, 1) @ bias

with G' = (h/2) * K^{-1} (rows 0 and n-1 zeroed) and
bias = sin(pi_f32) * K^{-1}[n-1, :].  Inputs are rounded to bf16 on host
(PSUM accumulates fp32): measured 2.3e-3 relative error vs the f32
reference solve, ~8x inside the 2e-2 gate, and it halves both the input
DMA bytes and the PE passes (fp32 matmul = 2 LOW/HIGH passes; bf16 = 1).

Device kernel.  Every core computes 64 distinct output columns,
out_blk = ftx.T @ gpx_blk, as 4 accumulating PE matmuls [K=128, M=128,
N=64] into one PSUM tile.  The bias row rides free: row j=0 of G' is
zero, so host-side ftx[0, :] = 1 and gpx[0, :] = bias.

DMA plan.  ft and gp are packed into ONE [128, 768] bf16 SBUF operand,
column-split across the two HWDGE rings (SP and Activation — the only
hardware rings): each ring posts a [128, 384]-column half covering all
128 partitions, because the partition<->SDMA-engine swizzle means a
64-partition DMA only engages half the 16 SDMA engines (measured 68
GB/s/ring row-split vs full-width column-split).  The halves are
chunk-aligned (half r = k-chunks 2r,2r+1 of both ft and gp) so the
first two accumulating matmuls start as soon as half 0 lands.

    scalar : DMA src half 0 -> wait copy -> DMA out rows 64:128
    sync   : DMA src half 1 -> wait copy -> DMA out rows 0:64
    tensor : warmup matmuls (keep the PE HAM clock un-throttled through
             the DMA window), wait half sems, 4 accumulating matmuls
    vector : PSUM -> SBUF copy (DMA cannot read PSUM; GpSimd cannot
             either, and the Act engine's first ACTIVATE pays a ~1.3us
             ACT_TABLE_LOAD and races its own ring's DMA post)

Also skipped (framework emissions this kernel never needs): the
const-AP memsets, the post-init all-engine barrier, the Block-exit
all-engine barrier (drain/gather/release — the NRT program wrapper's
own teardown barrier orders engine halt), and the partition_id
ExternalInput (enable_partition_id=False; this kernel is pure SPMD over
per-core input data).  The final DMA receipt is observed by the host
tens of microseconds after the last engine halts regardless (verified
bit-exact over repeated soak runs).
"""

import numpy as np
import ml_dtypes

import concourse.bass as bass
import concourse.mybir as mybir
from concourse import bass_utils

N = 512
B = 128
NCORES = 8
COLS = N // NCORES  # 64 output columns per core
AMPLITUDE = 0.1
F32 = mybir.dt.float32
BF16 = mybir.dt.bfloat16
HALF = 2 * B + 2 * COLS  # 256 ft cols + 128 gp cols = 384 per half
WARMUP = 14

_cache = {}


def _host_constants():
    h = 1.0 / (N - 1)
    c = AMPLITUDE / h
    main = np.full(N, 2.0 * c)
    main[0] = main[-1] = 1.0
    off = np.full(N - 1, -c)
    off[0] = off[-1] = 0.0
    K = np.diag(main) + np.diag(off, 1) + np.diag(off, -1)
    G = np.linalg.inv(K)  # float64
    Gp = G * (h / 2.0)
    Gp[0, :] = 0.0   # f[:,0] is the BC value, not forcing[:,0]
    Gp[-1, :] = 0.0  # f[:,-1] is the BC value, not forcing[:,-1]
    u_right = float(np.sin(np.float32(np.pi), dtype=np.float32))
    bias = u_right * G[N - 1, :]

    packs = []
    for core in range(NCORES):
        blk = Gp[:, core * COLS : (core + 1) * COLS].copy()  # [512, 64]
        blk[0, :] = bias[core * COLS : (core + 1) * COLS]  # ones-row bias fold
        # [p, t*COLS + i] = blk[t*128 + p, i]
        pk = blk.reshape(4, 128, COLS).transpose(1, 0, 2).reshape(128, 4 * COLS)
        packs.append(pk.astype(ml_dtypes.bfloat16))
    return packs


def _build_program():
    # Skip framework instructions this kernel never needs: const-AP
    # memsets (never read here) and every all-engine barrier (the one
    # from Bass.__init__ and the Block-exit drain/gather/release; the
    # NRT program wrapper has its own teardown barrier, and all
    # cross-engine ordering inside the kernel flows through its own
    # semaphores).  Patches are restored immediately after construction.
    patches = [
        (bass.BassEitherVectorEngine, "memset", lambda self, ap, c: None),
        (bass.Bass, "all_engine_barrier", lambda self, sem_only=False: None),
        (bass.BassEngine, "preamble", lambda self: None),
    ]
    saved = [(cls, name, getattr(cls, name)) for cls, name, _ in patches]
    for cls, name, fn in patches:
        setattr(cls, name, fn)
    try:
        nc = bass.Bass(
            "TRN2",
            target_bir_lowering=False,
            debug=False,
            enable_asserts=False,
            enable_partition_id=False,
            monotonic_sem_count=0,
        )

        src_d = nc.dram_tensor("src", [128, 2 * HALF], BF16, kind="ExternalInput")
        out_d = nc.dram_tensor("out", [B, COLS], F32, kind="ExternalOutput")

        with (
            nc.sbuf_tensor("src_sb", [128, 2 * HALF], BF16) as src_sb,
            nc.sbuf_tensor("out_sb", [B, COLS], F32) as out_sb,
            nc.sbuf_tensor("warm_sb", [128, COLS], BF16) as warm_sb,
            nc.psum_tensor("ps", [B, COLS], F32) as ps,
            nc.psum_tensor("warm_ps", [1, COLS], F32) as warm_ps,
            nc.semaphore("h0_sem") as h0_sem,
            nc.semaphore("h1_sem") as h1_sem,
            nc.semaphore("cpA_sem") as cpA_sem,
            nc.semaphore("cpB_sem") as cpB_sem,
            nc.semaphore("mm_sem") as mm_sem,
            nc.semaphore("out_sem") as out_sem,
        ):
            # No nc.Block(): instructions are emitted straight into the
            # entry basic block (each engine's sequencer executes its own
            # stream in program order) — this drops the per-engine
            # branch into block bodies and the block machinery entirely.
            #
            # Unbalanced 576/192-column split: the Act ring's post starts
            # ~750ns before the SP ring's (SP's wrapper preamble has a
            # ~700ns drain), so Act carries 3/4 of the bytes to equalize
            # the two completion times.  Column layout (bf16):
            #   0:128 ft0 | 128:256 ft1 | 256:320 gp0 | 320:384 gp1 |
            #   384:512 ft2 | 512:576 gp2 || 576:704 ft3 | 704:768 gp3
            nc.scalar.dma_start(src_sb[:, 0:576], src_d[:, 0:576]).then_inc(
                h0_sem, 16
            )
            nc.sync.dma_start(src_sb[:, 576:768], src_d[:, 576:768]).then_inc(
                h1_sem, 16
            )

            # Dummy matmuls on scratch data while the input DMAs are in
            # flight: sustains PE activity so the HAM clock gate reaches
            # full rate before the real matmuls.
            for _ in range(WARMUP):
                nc.tensor.matmul(
                    warm_ps[:, :], warm_sb[:, 0:1], warm_sb[:, :],
                    start=True, stop=True,
                )
            FT_OFF = (0, 128, 384, 576)
            GP_OFF = (256, 320, 512, 704)
            mm = None
            for t in range(4):
                if t == 0:
                    nc.tensor.wait_ge(h0_sem, 16)
                elif t == 3:
                    nc.tensor.wait_ge(h1_sem, 16)
                mm = nc.tensor.matmul(
                    ps[:, :],
                    src_sb[:, FT_OFF[t] : FT_OFF[t] + 128],
                    src_sb[:, GP_OFF[t] : GP_OFF[t] + COLS],
                    start=(t == 0),
                    stop=(t == 3),
                )
            mm.then_inc(mm_sem)

            # The measured NEFF window ends at the out-DMA's completion
            # semaphore, so the tail is pipelined: two half-row copies,
            # each gating an out post on its own ring, so the two 16KB
            # transfers (and their receipt chains) run concurrently.
            nc.vector.wait_ge(mm_sem, 1)
            nc.vector.tensor_copy(out_sb[0:64, :], ps[0:64, :]).then_inc(cpA_sem)
            nc.vector.tensor_copy(out_sb[64:128, :], ps[64:128, :]).then_inc(
                cpB_sem
            )

            nc.sync.wait_ge(cpA_sem, 1)
            nc.sync.dma_start(out_d[0:64, :], out_sb[0:64, :]).then_inc(
                out_sem, 16
            )
            nc.scalar.wait_ge(cpB_sem, 1)
            nc.scalar.dma_start(out_d[64:128, :], out_sb[64:128, :]).then_inc(
                out_sem, 16
            )

        nc.finalize()
    finally:
        for cls, name, fn in saved:
            setattr(cls, name, fn)
    return nc


def _get_state():
    if "state" not in _cache:
        _cache["state"] = (_build_program(), _host_constants())
    return _cache["state"]


def kernel(forcing_functions: np.ndarray, _trace: bool = False):
    nc, packs = _get_state()
    forcing = np.ascontiguousarray(forcing_functions, dtype=np.float32)
    ftx = forcing.T.copy()  # [512, 128]
    ftx[0, :] = 1.0  # ones row pairs with the bias row of gp
    # SBUF ft layout [p, t*128 + b] = ftx[t*128 + p, b]
    ft = (
        ftx.reshape(4, 128, B).transpose(1, 0, 2).reshape(128, 4 * B)
    ).astype(ml_dtypes.bfloat16)
    in_maps = []
    for c in range(NCORES):
        gp = packs[c]
        # cols 0:128 ft0 | 128:256 ft1 | 256:320 gp0 | 320:384 gp1 |
        #   384:512 ft2 | 512:576 gp2 | 576:704 ft3 | 704:768 gp3
        src = np.concatenate(
            [
                ft[:, 0:256],
                gp[:, 0:128],
                ft[:, 256:384],
                gp[:, 128:192],
                ft[:, 384:512],
                gp[:, 192:256],
            ],
            axis=1,
        )
        in_maps.append({"src": np.ascontiguousarray(src)})
    last_exc = None
    for _attempt in range(3):
        try:
            res = bass_utils.run_bass_kernel_spmd(
                nc, in_maps, core_ids=list(range(NCORES)), trace=_trace
            )
            break
        except Exception as exc:  # transient NRT/device flakes: retry
            last_exc = exc
            import time as _time

            _time.sleep(2.0)
    else:
        raise last_exc
    out = np.concatenate([r["out"] for r in res.results], axis=1)
    if _trace:
        return out, res
    return out


# revision 23
# speedup vs baseline: 1.0203x; 1.0203x over previous
"""Batched 1D Darcy solver (tridiagonal K shared across the batch) on 8
Trainium2 NeuronCores.

Math.  The reference assembles a CONSTANT tridiagonal matrix K (it depends
only on n=512 and AMPLITUDE=0.1) and solves K u = f where the RHS
f = assemble(forcing) is affine in the input:
    f[:, 1:-1] = forcing[:, 1:-1] * h/2,  f[:, 0] = 0,  f[:, -1] = sin(pi_f32)
Because K is constant, the whole solve collapses to one affine map,
precomputed on host in float64:

    u = forcing @ G' + ones(B, 1) @ bias

with G' = (h/2) * K^{-1} (rows 0 and n-1 zeroed) and
bias = sin(pi_f32) * K^{-1}[n-1, :].  Inputs are rounded to bf16 on host
(PSUM accumulates fp32): measured 2.3e-3 relative error vs the f32
reference solve, ~8x inside the 2e-2 gate, and it halves both the input
DMA bytes and the PE passes (fp32 matmul = 2 LOW/HIGH passes; bf16 = 1).

Device kernel.  Every core computes 64 distinct output columns,
out_blk = ftx.T @ gpx_blk, as 4 accumulating PE matmuls [K=128, M=128,
N=64] into one PSUM tile.  The bias row rides free: row j=0 of G' is
zero, so host-side ftx[0, :] = 1 and gpx[0, :] = bias.

DMA plan.  ft and gp are packed into ONE [128, 768] bf16 SBUF operand,
column-split across the two HWDGE rings (SP and Activation — the only
hardware rings): each ring posts a [128, 384]-column half covering all
128 partitions, because the partition<->SDMA-engine swizzle means a
64-partition DMA only engages half the 16 SDMA engines (measured 68
GB/s/ring row-split vs full-width column-split).  The halves are
chunk-aligned (half r = k-chunks 2r,2r+1 of both ft and gp) so the
first two accumulating matmuls start as soon as half 0 lands.

    scalar : DMA src half 0 -> wait copy -> DMA out rows 64:128
    sync   : DMA src half 1 -> wait copy -> DMA out rows 0:64
    tensor : warmup matmuls (keep the PE HAM clock un-throttled through
             the DMA window), wait half sems, 4 accumulating matmuls
    vector : PSUM -> SBUF copy (DMA cannot read PSUM; GpSimd cannot
             either, and the Act engine's first ACTIVATE pays a ~1.3us
             ACT_TABLE_LOAD and races its own ring's DMA post)

Also skipped (framework emissions this kernel never needs): the
const-AP memsets, the post-init all-engine barrier, the Block-exit
all-engine barrier (drain/gather/release — the NRT program wrapper's
own teardown barrier orders engine halt), and the partition_id
ExternalInput (enable_partition_id=False; this kernel is pure SPMD over
per-core input data).  The final DMA receipt is observed by the host
tens of microseconds after the last engine halts regardless (verified
bit-exact over repeated soak runs).
"""

import numpy as np
import ml_dtypes

import concourse.bass as bass
import concourse.mybir as mybir
from concourse import bass_utils

N = 512
B = 128
NCORES = 8
COLS = N // NCORES  # 64 output columns per core
AMPLITUDE = 0.1
F32 = mybir.dt.float32
BF16 = mybir.dt.bfloat16
HALF = 2 * B + 2 * COLS  # 256 ft cols + 128 gp cols = 384 per half
WARMUP = 14

_cache = {}


def _host_constants():
    h = 1.0 / (N - 1)
    c = AMPLITUDE / h
    main = np.full(N, 2.0 * c)
    main[0] = main[-1] = 1.0
    off = np.full(N - 1, -c)
    off[0] = off[-1] = 0.0
    K = np.diag(main) + np.diag(off, 1) + np.diag(off, -1)
    G = np.linalg.inv(K)  # float64
    Gp = G * (h / 2.0)
    Gp[0, :] = 0.0   # f[:,0] is the BC value, not forcing[:,0]
    Gp[-1, :] = 0.0  # f[:,-1] is the BC value, not forcing[:,-1]
    u_right = float(np.sin(np.float32(np.pi), dtype=np.float32))
    bias = u_right * G[N - 1, :]

    packs = []
    for core in range(NCORES):
        blk = Gp[:, core * COLS : (core + 1) * COLS].copy()  # [512, 64]
        blk[0, :] = bias[core * COLS : (core + 1) * COLS]  # ones-row bias fold
        # [p, t*COLS + i] = blk[t*128 + p, i]
        pk = blk.reshape(4, 128, COLS).transpose(1, 0, 2).reshape(128, 4 * COLS)
        packs.append(pk.astype(ml_dtypes.bfloat16))
    return packs


def _build_program():
    # Skip framework instructions this kernel never needs: const-AP
    # memsets (never read here) and every all-engine barrier (the one
    # from Bass.__init__ and the Block-exit drain/gather/release; the
    # NRT program wrapper has its own teardown barrier, and all
    # cross-engine ordering inside the kernel flows through its own
    # semaphores).  Patches are restored immediately after construction.
    patches = [
        (bass.BassEitherVectorEngine, "memset", lambda self, ap, c: None),
        (bass.Bass, "all_engine_barrier", lambda self, sem_only=False: None),
        (bass.BassEngine, "preamble", lambda self: None),
    ]
    saved = [(cls, name, getattr(cls, name)) for cls, name, _ in patches]
    for cls, name, fn in patches:
        setattr(cls, name, fn)
    try:
        nc = bass.Bass(
            "TRN2",
            target_bir_lowering=False,
            debug=False,
            enable_asserts=False,
            enable_partition_id=False,
            monotonic_sem_count=0,
        )

        src_d = nc.dram_tensor("src", [128, 2 * HALF], BF16, kind="ExternalInput")
        # Output leaves transposed ([COLS, B] = out.T): the matmuls emit
        # psT = gp.T @ ft directly (swapped operands), so the staging
        # tile is [64, 512B] — the out DMA needs only 64 fat descriptors
        # and the host transposes back for free.
        out_d = nc.dram_tensor("out", [COLS, B], F32, kind="ExternalOutput")

        with (
            nc.sbuf_tensor("src_sb", [128, 2 * HALF], BF16) as src_sb,
            nc.sbuf_tensor("out_sb", [COLS, B], F32) as out_sb,
            nc.sbuf_tensor("warm_sb", [128, COLS], BF16) as warm_sb,
            nc.psum_tensor("ps", [COLS, B], F32) as ps,
            nc.psum_tensor("warm_ps", [1, COLS], F32) as warm_ps,
            nc.semaphore("h0_sem") as h0_sem,
            nc.semaphore("h1_sem") as h1_sem,
            nc.semaphore("cpA_sem") as cpA_sem,
            nc.semaphore("mm_sem") as mm_sem,
            nc.semaphore("out_sem") as out_sem,
        ):
            # No nc.Block(): instructions are emitted straight into the
            # entry basic block (each engine's sequencer executes its own
            # stream in program order) — this drops the per-engine
            # branch into block bodies and the block machinery entirely.
            #
            # Unbalanced 576/192-column split: the Act ring's post starts
            # ~750ns before the SP ring's (SP's wrapper preamble has a
            # ~700ns drain), so Act carries 3/4 of the bytes to equalize
            # the two completion times.  Column layout (bf16):
            #   0:128 ft0 | 128:256 ft1 | 256:320 gp0 | 320:384 gp1 |
            #   384:512 ft2 | 512:576 gp2 || 576:704 ft3 | 704:768 gp3
            nc.scalar.dma_start(src_sb[:, 0:576], src_d[:, 0:576]).then_inc(
                h0_sem, 16
            )
            nc.sync.dma_start(src_sb[:, 576:768], src_d[:, 576:768]).then_inc(
                h1_sem, 16
            )

            # Dummy matmuls on scratch data while the input DMAs are in
            # flight: sustains PE activity so the HAM clock gate reaches
            # full rate before the real matmuls.
            for _ in range(WARMUP):
                nc.tensor.matmul(
                    warm_ps[:, :], warm_sb[:, 0:1], warm_sb[:, :],
                    start=True, stop=True,
                )
            FT_OFF = (0, 128, 384, 576)
            GP_OFF = (256, 320, 512, 704)
            mm = None
            for t in range(4):
                if t == 0:
                    nc.tensor.wait_ge(h0_sem, 16)
                elif t == 3:
                    nc.tensor.wait_ge(h1_sem, 16)
                mm = nc.tensor.matmul(
                    ps[:, :],
                    src_sb[:, GP_OFF[t] : GP_OFF[t] + COLS],
                    src_sb[:, FT_OFF[t] : FT_OFF[t] + 128],
                    start=(t == 0),
                    stop=(t == 3),
                )
            mm.then_inc(mm_sem)

            # The measured NEFF window ends at the out-DMA's last
            # completion receipt, so the tail is one short copy plus one
            # 64-descriptor post on the SP ring (a second ring doubles
            # the receipt count and stretches the receipt straggle).
            nc.vector.wait_ge(mm_sem, 1)
            nc.vector.tensor_copy(out_sb[:], ps[:, :]).then_inc(cpA_sem)

            nc.sync.wait_ge(cpA_sem, 1)
            nc.sync.dma_start(out_d[:, :], out_sb[:]).then_inc(out_sem, 16)

        nc.finalize()
    finally:
        for cls, name, fn in saved:
            setattr(cls, name, fn)
    return nc


def _get_state():
    if "state" not in _cache:
        _cache["state"] = (_build_program(), _host_constants())
    return _cache["state"]


def kernel(forcing_functions: np.ndarray, _trace: bool = False):
    nc, packs = _get_state()
    forcing = np.ascontiguousarray(forcing_functions, dtype=np.float32)
    ftx = forcing.T.copy()  # [512, 128]
    ftx[0, :] = 1.0  # ones row pairs with the bias row of gp
    # SBUF ft layout [p, t*128 + b] = ftx[t*128 + p, b]
    ft = (
        ftx.reshape(4, 128, B).transpose(1, 0, 2).reshape(128, 4 * B)
    ).astype(ml_dtypes.bfloat16)
    in_maps = []
    for c in range(NCORES):
        gp = packs[c]
        # cols 0:128 ft0 | 128:256 ft1 | 256:320 gp0 | 320:384 gp1 |
        #   384:512 ft2 | 512:576 gp2 | 576:704 ft3 | 704:768 gp3
        src = np.concatenate(
            [
                ft[:, 0:256],
                gp[:, 0:128],
                ft[:, 256:384],
                gp[:, 128:192],
                ft[:, 384:512],
                gp[:, 192:256],
            ],
            axis=1,
        )
        in_maps.append({"src": np.ascontiguousarray(src)})
    last_exc = None
    for _attempt in range(3):
        try:
            res = bass_utils.run_bass_kernel_spmd(
                nc, in_maps, core_ids=list(range(NCORES)), trace=_trace
            )
            break
        except Exception as exc:  # transient NRT/device flakes: retry
            last_exc = exc
            import time as _time

            _time.sleep(2.0)
    else:
        raise last_exc
    out = np.concatenate([r["out"].T for r in res.results], axis=1)
    if _trace:
        return out, res
    return out
